# revision 19
# baseline (speedup 1.0000x reference)
"""Trainium2 Bass kernel for chunked (block-causal) attention.

Problem: B=1, S=4096, D=1024, H=16 heads x 64 dim, chunk=1024.
  q,k,v = x@wq+bq, x@wk+bk, x@wv+bv  (per-head split)
  scores = q k^T / 8, block-causal mask at chunk granularity, softmax
  out = (attn @ v) recombined, then @ wo + bo

Sharding (8 NeuronCores): head-parallel attention (2 heads per core) +
sequence-parallel output projection. Each core computes Q/K/V for its 2
heads over the full sequence, block-sparse attention (only the allowed
chunk-pairs are ever computed -> mask is structural, no -1e9 needed),
then an AllToAll exchanges attention outputs so core c owns all 16 heads
for query rows [512c, 512c+512); each core runs the full out-projection
for its row slice. Softmax normalization (divide by Z) happens after the
AllToAll on the receiver (Z rows travel with the payload) so the
reciprocal runs once on a [16, 512] tile instead of many [1, N] strips.

All matmuls run as float32r (full-rate fp32, ~1.4e-4 rel err measured).
"""
import sys

for _p in ("/opt/trn_rl_repo",):
    if _p not in sys.path:
        sys.path.insert(0, _p)

import numpy as np
import concourse.bass as bass  # noqa: F401  (engine types referenced via nc)
import concourse.mybir as mybir
import concourse.tile as tile
from concourse import bacc
from concourse.bass_utils import run_bass_kernel_spmd
from concourse.masks import make_identity

f32 = mybir.dt.float32
f32r = mybir.dt.float32r
AF = mybir.ActivationFunctionType

NCORES = 8
S = 4096
D = 1024
H = 16
HD = 64
CHUNK = 1024
SCALE = HD ** -0.5
QSLICE = S // NCORES          # 512 query rows owned per core after A2A
NSC = 8                       # seq chunks of 512 for QKV projection
SC = S // NSC                 # 512
NKC = S // 128                # 32 k-chunks of 128
NSPAN = S // CHUNK            # 4 q-spans of 1024 (== chunks)


def build(with_collective=True, num_devices=NCORES):
    nc = bacc.Bacc("TRN2", target_bir_lowering=False, debug=False,
                   num_devices=num_devices)

    xT_ext = nc.dram_tensor("xT", [D, S], f32, kind="ExternalInput")
    wq_ext = nc.dram_tensor("wq", [D, 128], f32, kind="ExternalInput")
    wk_ext = nc.dram_tensor("wk", [D, 128], f32, kind="ExternalInput")
    wv_ext = nc.dram_tensor("wv", [D, 128], f32, kind="ExternalInput")
    bq_ext = nc.dram_tensor("bq", [128], f32, kind="ExternalInput")
    bk_ext = nc.dram_tensor("bk", [128], f32, kind="ExternalInput")
    bv_ext = nc.dram_tensor("bv", [128], f32, kind="ExternalInput")
    wo_ext = nc.dram_tensor("wo", [D, D], f32, kind="ExternalInput")
    bo_ext = nc.dram_tensor("bo", [D], f32, kind="ExternalInput")
    sel_ext = nc.dram_tensor("sel", [16, NCORES, 128], f32, kind="ExternalInput")
    y_ext = nc.dram_tensor("y", [QSLICE, D], f32, kind="ExternalOutput")

    with tile.TileContext(nc) as tc:
        with (
            tc.tile_pool(name="consts", bufs=1) as consts,
            tc.tile_pool(name="work", bufs=1) as work,
            tc.tile_pool(name="pt", bufs=2) as ptp,
            tc.tile_pool(name="psum", bufs=1, space="PSUM") as psum,
            tc.tile_pool(name="dram", bufs=1, space="DRAM") as dram,
        ):
            # ---- constants / weights ----
            ident = consts.tile([128, 128], f32, tag="ident")
            make_identity(nc, ident)
            ones_t = consts.tile([128, 1], f32, tag="ones")
            nc.gpsimd.memset(ones_t, 1.0)

            wq_t = consts.tile([128, 8, 128], f32r, tag="wq")
            wk_t = consts.tile([128, 8, 128], f32r, tag="wk")
            wv_t = consts.tile([128, 8, 128], f32r, tag="wv")
            bq_t = consts.tile([128, 1], f32, tag="bq")
            bk_t = consts.tile([128, 1], f32, tag="bk")
            bv_t = consts.tile([128, 1], f32, tag="bv")
            wo_t = consts.tile([128, 8, D], f32r, tag="wo")
            bo_row = consts.tile([1, D], f32r, tag="bo_row")
            sel = consts.tile([16, NCORES, 128], f32, tag="sel")
            nc.sync.dma_start(out=sel, in_=sel_ext[:])
            onesr = consts.tile([1, 128], f32r, tag="onesr")
            nc.vector.tensor_copy(onesr, ones_t[0:1, 0:1].broadcast_to([1, 128]))

            # ---- working tensors (live through attention + a2a send) ----
            QT = work.tile([128, S], f32r, tag="QT")    # [2h*64, S]
            KT = work.tile([128, S], f32r, tag="KT")
            VT = work.tile([128, S], f32, tag="VT")
            va = work.tile([128, NKC, 130], f32r, tag="va")  # V blocks + ones cols

            # weight loads (emitted after nothing - first in DMA order after xt0
            # via scheduler priorities; keep them compact)
            for w_t, w_ext2 in ((wq_t, wq_ext), (wk_t, wk_ext), (wv_t, wv_ext)):
                nc.sync.dma_start(
                    out=w_t,
                    in_=w_ext2[:].rearrange("(dc p) m -> p dc m", p=128).bitcast(f32r))
            nc.sync.dma_start(out=bq_t, in_=bq_ext[:].unsqueeze(1))
            nc.sync.dma_start(out=bk_t, in_=bk_ext[:].unsqueeze(1))
            nc.sync.dma_start(out=bv_t, in_=bv_ext[:].unsqueeze(1))

            # ---- QKV projection: out^T layout [cols, seq] ----
            # ones columns of va (indices 64 and 129 per k-chunk)
            nc.vector.tensor_copy(
                va[:, :, 64:65], ones_t.unsqueeze(1).broadcast_to([128, NKC, 1]))
            nc.vector.tensor_copy(
                va[:, :, 129:130], ones_t.unsqueeze(1).broadcast_to([128, NKC, 1]))
            with tc.tile_pool(name="xtp", bufs=2) as xtp:
                for sc in range(NSC):
                    xt = xtp.tile([128, 8, SC], f32r, tag="xt")
                    for dc in range(8):
                        nc.sync.dma_start(
                            out=xt[:, dc:dc + 1, :],
                            in_=xT_ext[128 * dc:128 * dc + 128,
                                       sc * SC:(sc + 1) * SC]
                                .rearrange("(dc p) n -> p dc n", p=128)
                                .bitcast(f32r))
                    for w_t, b_t, OUT in ((wq_t, bq_t, QT), (wk_t, bk_t, KT),
                                          (wv_t, bv_t, VT)):
                        ps = psum.tile([128, SC], f32, tag="acc", bufs=2)
                        for dc in range(8):
                            nc.tensor.matmul(ps, w_t[:, dc, :], xt[:, dc, :],
                                             start=(dc == 0), stop=(dc == 7))
                        nc.vector.tensor_scalar_add(
                            OUT[:, sc * SC:(sc + 1) * SC], ps, b_t)
                    # V^T -> V blocks for the chunk range just produced
                    for kc in range(4 * sc, 4 * sc + 4):
                        tp = psum.tile([128, 128], f32, tag="acc", bufs=2)
                        nc.tensor.transpose(
                            tp, VT[:, kc * 128:(kc + 1) * 128], ident)
                        nc.vector.tensor_copy(va[:, kc, 0:64], tp[:, 0:64])
                        nc.vector.tensor_copy(va[:, kc, 65:129], tp[:, 64:128])

            # ---- a2a buffers (shard j = 2 heads + Z rows for q-slice j),
            #      split into two q-halves for collective/post overlap ----
            a2a_in = [dram.tile([NCORES, 130, QSLICE // 2], f32, tag="a2ain",
                                name=f"a2ain{i}") for i in range(2)]
            a2a_out = [dram.tile([NCORES, 130, QSLICE // 2], f32, tag="a2aout",
                                 name=f"a2aout{i}") for i in range(2)]

            # ---- block-sparse attention (structural mask) ----
            for span in range(NSPAN):
                q0 = span * CHUNK
                nkc = (span + 1) * 8
                for h in range(2):
                    hq = QT[64 * h:64 * h + 64, :]
                    hk = KT[64 * h:64 * h + 64, :]
                    pvh = [psum.tile([65, 512], f32, tag="pv", bufs=2,
                                     name=f"pvh{span}_{h}_{jj}")
                           for jj in range(2)]
                    for kc in range(nkc):
                        sps = psum.tile([128, CHUNK], f32, tag="sps", bufs=2)
                        for j in range(2):
                            nc.tensor.matmul(
                                sps[:, 512 * j:512 * j + 512],
                                hk[:, kc * 128:(kc + 1) * 128],
                                hq[:, q0 + 512 * j:q0 + 512 * j + 512],
                                start=True, stop=True)
                        pt = ptp.tile([128, CHUNK], f32r, tag="pt")
                        nc.scalar.activation(pt, sps, AF.Exp, scale=SCALE)
                        for j in range(2):
                            nc.tensor.matmul(
                                pvh[j],
                                va[:, kc, 65 * h:65 * h + 65],
                                pt[:, 512 * j:512 * j + 512],
                                start=(kc == 0), stop=(kc == nkc - 1))
                    apv = ptp.tile([65, CHUNK], f32, tag="apv")
                    for jj in range(2):
                        nc.vector.tensor_copy(
                            apv[:, 512 * jj:512 * jj + 512], pvh[jj])
                        j = 2 * span + jj
                        for hf in range(2):
                            c0 = 512 * jj + 256 * hf
                            nc.sync.dma_start(
                                out=a2a_in[hf][j, 64 * h:64 * h + 64, :],
                                in_=apv[0:64, c0:c0 + 256])
                            nc.sync.dma_start(
                                out=a2a_in[hf][j, 128 + h:129 + h, :],
                                in_=apv[64:65, c0:c0 + 256])

            # ---- AllToAll (two halves) ----
            for hf in range(2):
                if with_collective:
                    nc.gpsimd.collective_compute(
                        "AllToAll",
                        mybir.AluOpType.bypass,
                        replica_groups=[list(range(NCORES))],
                        ins=[a2a_in[hf].opt()],
                        outs=[a2a_out[hf].opt()],
                    )
                else:
                    nc.sync.dma_start(out=a2a_out[hf][:], in_=a2a_in[hf][:])

            # ---- receiver: normalize by 1/Z, full out-projection ----
            nc.sync.dma_start(
                out=wo_t,
                in_=wo_ext[:].rearrange("(pc p) d -> p pc d", p=128).bitcast(f32r))
            nc.sync.dma_start(out=bo_row, in_=bo_ext[:].unsqueeze(0).bitcast(f32r))
            with tc.tile_pool(name="post", bufs=1) as post:
                HQ = QSLICE // 2
                y_sbs = [post.tile([128, D], f32, tag="ysb", bufs=4,
                                   name=f"ysb{i}")
                         for i in range(4)]
                for hf in range(2):
                    raw = post.tile([128, 8, HQ], f32r, tag="raw", bufs=2,
                                    name=f"raw{hf}")
                    for r in range(NCORES):
                        nc.sync.dma_start(
                            out=raw[:, r:r + 1, :],
                            in_=a2a_out[hf][r:r + 1, 0:128, :]
                                .rearrange("r p n -> p r n").bitcast(f32r))
                    zc = post.tile([16, HQ], f32, tag="zc", bufs=2,
                                   name=f"zc{hf}")
                    for r in range(NCORES):
                        nc.gpsimd.dma_start(
                            out=zc[2 * r:2 * r + 2, :],
                            in_=a2a_out[hf][r, 128:130, :])
                    rz = post.tile([16, HQ], f32, tag="rz", bufs=2,
                                   name=f"rz{hf}")
                    nc.vector.reciprocal(rz, zc)
                    # sel[z, r, p] = 1 iff z == 2r + p//64 ; bcast = sel_r.T @ rz
                    for r in range(NCORES):
                        bcp = psum.tile([128, HQ], f32, tag="acc", bufs=2)
                        nc.tensor.matmul(bcp, sel[:, r, :], rz,
                                         start=True, stop=True)
                        sl = raw[:, r, :]
                        nc.vector.tensor_mul(sl, sl.bitcast(f32), bcp)
                    for qq in range(2 * hf, 2 * hf + 2):
                        qs = slice((qq - 2 * hf) * 128, (qq - 2 * hf) * 128 + 128)
                        for dh in range(2):
                            dsl = slice(512 * dh, 512 * dh + 512)
                            ps = psum.tile([128, 512], f32, tag="acc", bufs=2)
                            for pc in range(8):
                                nc.tensor.matmul(
                                    ps,
                                    raw[:, pc, qs],
                                    wo_t[:, pc, dsl],
                                    start=(pc == 0), stop=False)
                            nc.tensor.matmul(
                                ps, onesr, bo_row[:, dsl],
                                start=False, stop=True)
                            nc.vector.tensor_copy(y_sbs[qq][:, dsl], ps)
                        nc.sync.dma_start(
                            out=y_ext[qq * 128:(qq + 1) * 128, :], in_=y_sbs[qq])

    nc.compile()
    return nc


SEL = np.zeros((16, NCORES, 128), dtype=np.float32)
for _r in range(NCORES):
    for _h in range(2):
        SEL[2 * _r + _h, _r, 64 * _h:64 * _h + 64] = 1.0

_NC = None


def _get_nc():
    global _NC
    if _NC is None:
        _NC = build()
    return _NC


def _run(inputs, trace=False):
    x = np.asarray(inputs["x"], dtype=np.float32)
    wq = np.asarray(inputs["wq"], dtype=np.float32)
    wk = np.asarray(inputs["wk"], dtype=np.float32)
    wv = np.asarray(inputs["wv"], dtype=np.float32)
    bq = np.asarray(inputs["bq"], dtype=np.float32)
    bk = np.asarray(inputs["bk"], dtype=np.float32)
    bv = np.asarray(inputs["bv"], dtype=np.float32)
    wo = np.asarray(inputs["wo"], dtype=np.float32)
    bo = np.asarray(inputs["bo"], dtype=np.float32)

    B = x.shape[0]
    assert x.shape == (B, S, D) and B == 1

    xT = np.ascontiguousarray(x.reshape(S, D).T)
    wo_c = np.ascontiguousarray(wo)
    in_maps = []
    for c in range(NCORES):
        lo = 128 * c
        in_maps.append({
            "xT": xT,
            "wq": np.ascontiguousarray(wq[:, lo:lo + 128]),
            "wk": np.ascontiguousarray(wk[:, lo:lo + 128]),
            "wv": np.ascontiguousarray(wv[:, lo:lo + 128]),
            "bq": np.ascontiguousarray(bq[lo:lo + 128]),
            "bk": np.ascontiguousarray(bk[lo:lo + 128]),
            "bv": np.ascontiguousarray(bv[lo:lo + 128]),
            "wo": wo_c,
            "bo": bo,
            "sel": SEL,
        })

    nc = _get_nc()
    res = run_bass_kernel_spmd(nc, in_maps, core_ids=list(range(NCORES)),
                               trace=trace)
    y = np.concatenate([res.results[c]["y"] for c in range(NCORES)], axis=0)
    return y.reshape(B, S, D).astype(np.float32), res


def kernel(**inputs):
    y, _ = _run(inputs, trace=False)
    return y
